# revision 4
# baseline (speedup 1.0000x reference)
"""DenseGAT Trainium2 kernel (8 NeuronCores, batch-parallel), v2.

Math: per (batch, head):
  h = x @ W.T ; a_src[i] = h[i]*att_src ; a_dst[j] = h[j]*att_dst
  s_ij = a_src[i] + a_dst[j] ; P = adj * exp(leakyrelu_0.2(s))
  out[i] = (P @ h)[i] / sum_j P[i,j]

Identity: exp(lrelu(s)) = [s>=0]*u_i*v_j + [s<0]*p_i*q_j with
  u = exp(a_src), v = exp(a_dst), p = exp(0.2 a_src), q = exp(0.2 a_dst).
With adjH = adj * [s>=0], rv = v*[h|1], rq = q*[h|1]:
  out_aug = p .* (adjT@rq + adjH@(-rq)) + u .* (adjH@rv)
where col 64 of the aug matmuls is the softmax denominator.

v2 structure vs baseline:
  - host ships adj as bf16 -> adjT built by 16 DMA-xbar transposes straight
    from DRAM (no PE transposes, no PSUM evacuations)
  - host ships x pre-transposed (bf16) and W in both layouts
  - t-outer / cg-inner attention loop with all 16 po accumulators resident
    in PSUM (packed 3 per bank, 6 banks) -> adjH liveness is a small ring of
    full rows, masks are built with full-row [128, 2048] DVE ops
  - rq/rnv built with broadcast tensor_tensor on DVE instead of 48 tiny
    ACT ops per head

Each core handles one batch sample (B=8 across 8 cores).
"""

import numpy as np
import ml_dtypes

import concourse.bass as bass
import concourse.mybir as mybir
import concourse.tile as tile
from concourse import bacc
from concourse.bass_utils import run_bass_kernel_spmd
from concourse.masks import make_identity

P = 128
B, L, CIN, COUT, HEADS = 8, 2048, 256, 256, 4
HD = COUT // HEADS          # 64
NT = L // P                 # 16 tiles along L
KB = CIN // P               # 2 chunks along cin/cout
NEG = 0.2
N_CORES = 8
NAUG = HD + 1               # 65

F32 = mybir.dt.float32
BF16 = mybir.dt.bfloat16
AF = mybir.ActivationFunctionType
OP = mybir.AluOpType

_NC_CACHE = {}


def _build():
    nc = bacc.Bacc(None, target_bir_lowering=False, debug=False)
    adj_in = nc.declare_dram_parameter("adjbf", [L, L], BF16, isOutput=False)
    xt_in = nc.declare_dram_parameter("xTbf", [CIN, L], BF16, isOutput=False)
    w_in = nc.declare_dram_parameter("W", [COUT, CIN], F32, isOutput=False)
    wt_in = nc.declare_dram_parameter("wTbf", [CIN, COUT], BF16, isOutput=False)
    asrc_in = nc.declare_dram_parameter("att_src", [1, HEADS, 1, HD], F32, isOutput=False)
    adst_in = nc.declare_dram_parameter("att_dst", [1, HEADS, 1, HD], F32, isOutput=False)
    out_d = nc.declare_dram_parameter("out", [L, COUT], F32, isOutput=True)

    with tile.TileContext(nc) as tc:
        with (
            tc.tile_pool(name="const", bufs=1) as cpool,
            tc.tile_pool(name="big", bufs=1) as big,
        ):
            ident_f32 = cpool.tile([P, P], F32)
            make_identity(nc, ident_f32)
            ones_bf = cpool.tile([1, P], BF16)
            nc.vector.memset(ones_bf[:], 1.0)

            # persistent tensors
            adjT = big.tile([P, NT, L], BF16)       # adj^T: adjT[j, t, i] = adj[i, t*128+j]
            h_bf = big.tile([P, NT, COUT], BF16)    # h natural (L on partitions)
            a_bf = big.tile([8, L], BF16)           # 2H score rows
            a_cols = big.tile([P, NT, 8], F32)      # transposed score columns

            # ---------------- prep: x/W/scores/h ----------------
            with (
                tc.tile_pool(name="prep", bufs=1) as prep,
                tc.tile_pool(name="prep_ps", bufs=2, space="PSUM") as pps,
                tc.tile_pool(name="small_ps", bufs=2, space="PSUM") as sps,
            ):
                xT_bf = prep.tile([P, KB, L], BF16)
                wT_bf = prep.tile([P, KB, COUT], BF16)
                w_nat = prep.tile([P, KB, CIN], F32)
                attW = prep.tile([P, KB, 2 * HEADS], F32)
                attc = prep.tile([P, KB, 2 * HEADS], BF16)
                a_all = prep.tile([8, L], F32)

                nc.sync.dma_start(
                    out=xT_bf[:], in_=xt_in[:].rearrange("(kb p) l -> p kb l", p=P)
                )
                nc.sync.dma_start(
                    out=wT_bf[:], in_=wt_in[:].rearrange("(kb p) c -> p kb c", p=P)
                )
                nc.sync.dma_start(
                    out=w_nat[:], in_=w_in[:].rearrange("(kb p) c -> p kb c", p=P)
                )
                # adjacency transpose: straight from DRAM through the DMA xbar
                # (after the small prep DMAs so prep compute can start early)
                for c in range(NT):
                    nc.sync.dma_start_transpose(
                        out=adjT[:, :, c * P : (c + 1) * P],
                        in_=adj_in[c * P : (c + 1) * P, :],
                    )
                nc.vector.memset(attW[:], 0.0)
                for h in range(HEADS):
                    cb, prow = divmod(HD * h, P)
                    nc.sync.dma_start(
                        out=attW[prow : prow + HD, cb, 2 * h : 2 * h + 1],
                        in_=asrc_in[0, h, 0, :].rearrange("(d one) -> d one", one=1),
                    )
                    nc.sync.dma_start(
                        out=attW[prow : prow + HD, cb, 2 * h + 1 : 2 * h + 2],
                        in_=adst_in[0, h, 0, :].rearrange("(d one) -> d one", one=1),
                    )

                # attc = W^T @ attW : [cin, 2H] (bf16 for the score matmul)
                for mb in range(KB):
                    ap_ps = sps.tile([P, 2 * HEADS], F32, tag="small")
                    for cb in range(KB):
                        nc.tensor.matmul(
                            ap_ps[:], w_nat[:, cb, mb * P : (mb + 1) * P], attW[:, cb, :],
                            start=(cb == 0), stop=(cb == KB - 1),
                        )
                    nc.scalar.activation(attc[:, mb, :], ap_ps[:], AF.Copy, bias=0.0, scale=1.0)

                # a_all = attc^T @ xT : [2H, L]
                for nb in range(4):
                    a_ps = sps.tile([8, 512], F32, tag="small")
                    for kb in range(KB):
                        nc.tensor.matmul(
                            a_ps[:], attc[:, kb, :], xT_bf[:, kb, nb * 512 : (nb + 1) * 512],
                            start=(kb == 0), stop=(kb == KB - 1),
                        )
                    nc.scalar.activation(
                        a_all[:, nb * 512 : (nb + 1) * 512], a_ps[:], AF.Copy, bias=0.0, scale=1.0
                    )
                nc.vector.tensor_copy(a_bf[:], a_all[:])

                for t in range(NT):
                    acp = sps.tile([P, 8], F32, tag="small")
                    nc.tensor.transpose(
                        acp[:], a_all[0:8, t * P : (t + 1) * P], ident_f32[0:8, 0:8]
                    )
                    nc.scalar.activation(a_cols[:, t, :], acp[:], AF.Copy, bias=0.0, scale=1.0)

                # h = x @ W.T
                for c in range(NT):
                    hp = pps.tile([P, COUT], F32, tag="prep")
                    for kb in range(KB):
                        nc.tensor.matmul(
                            hp[:], xT_bf[:, kb, c * P : (c + 1) * P], wT_bf[:, kb, :],
                            start=(kb == 0), stop=(kb == KB - 1),
                        )
                    nc.scalar.activation(h_bf[:, c, :], hp[:], AF.Copy, bias=0.0, scale=1.0)

            # ---------------- main: per-head attention ----------------
            with (
                tc.tile_pool(name="cols", bufs=2) as colp,
                tc.tile_pool(name="rhs", bufs=2) as rhsp,
                tc.tile_pool(name="bc", bufs=2) as bcp,
                tc.tile_pool(name="ring", bufs=5) as ringp,
                tc.tile_pool(name="est", bufs=4) as estp,
                tc.tile_pool(name="sall", bufs=2) as sallp,
                tc.tile_pool(name="outst", bufs=2) as outp,
                tc.tile_pool(name="po_ps", bufs=1, space="PSUM") as pops,
                tc.tile_pool(name="bc_ps", bufs=2, space="PSUM") as bcps,
            ):
                po_big = pops.tile([P, 6, 512], F32)

                def po_slice(cg):
                    b, o = divmod(cg, 3)
                    return po_big[:, b, o * 130 : o * 130 + 130]

                def head_prep(h):
                    st = {}
                    asl = a_cols[:, :, 2 * h : 2 * h + 1].rearrange("p t one -> p (t one)")
                    adl = a_cols[:, :, 2 * h + 1 : 2 * h + 2].rearrange("p t one -> p (t one)")
                    # f32 cols for ACT scales / TS scalars
                    ucol = st["ucol"] = colp.tile([P, NT], F32, tag="ucol", name="ucol")
                    pcol = st["pcol"] = colp.tile([P, NT], F32, tag="pcol", name="pcol")
                    nadst = st["nadst"] = colp.tile([P, NT], F32, tag="nadst", name="nadst")
                    # bf16 cols for DVE broadcast multiplies
                    vcol = st["vcol"] = colp.tile([P, NT], BF16, tag="vcol", name="vcol")
                    qcol = st["qcol"] = colp.tile([P, NT], BF16, tag="qcol", name="qcol")
                    nqcol = st["nqcol"] = colp.tile([P, NT], BF16, tag="nqcol", name="nqcol")
                    nc.scalar.activation(ucol[:], asl, AF.Exp, bias=0.0, scale=1.0)
                    nc.scalar.activation(pcol[:], asl, AF.Exp, bias=0.0, scale=NEG)
                    nc.scalar.activation(vcol[:], adl, AF.Exp, bias=0.0, scale=1.0)
                    nc.scalar.activation(qcol[:], adl, AF.Exp, bias=0.0, scale=NEG)
                    nc.vector.tensor_scalar(
                        out=nadst[:], in0=adl, scalar1=-1.0, scalar2=None, op0=OP.mult
                    )
                    nc.vector.tensor_scalar(
                        out=nqcol[:], in0=qcol[:], scalar1=-1.0, scalar2=None, op0=OP.mult
                    )

                    # a_src broadcast tile [128, L] bf16 via rank-1 matmul
                    arow = bcp.tile([1, L], BF16, tag="arow")
                    nc.sync.dma_start(out=arow[:], in_=a_bf[2 * h : 2 * h + 1, :])
                    bcast = st["bcast"] = bcp.tile([P, L], BF16, tag="bcast", name="bcast")
                    for nb in range(4):
                        bps = bcps.tile([P, 512], F32, tag="bps")
                        nc.tensor.matmul(
                            bps[:], ones_bf[:], arow[0:1, nb * 512 : (nb + 1) * 512],
                            start=True, stop=True,
                        )
                        nc.scalar.activation(
                            bcast[:, nb * 512 : (nb + 1) * 512], bps[:], AF.Copy, bias=0.0, scale=1.0
                        )

                    # rhs tensors: rq = q*[h|1]; rnv = [-q*[h|1] | v*[h|1]] packed
                    rq = st["rq"] = rhsp.tile([P, NT, NAUG], BF16, tag="rq", name="rq")
                    rnv = st["rnv"] = rhsp.tile([P, NT, 2 * NAUG], BF16, tag="rnv", name="rnv")
                    hsl = h_bf[:, :, h * HD : (h + 1) * HD]
                    nc.vector.tensor_tensor(
                        out=rq[:, :, 0:HD], in0=hsl,
                        in1=qcol[:].unsqueeze(2).to_broadcast([P, NT, HD]), op=OP.mult,
                    )
                    nc.vector.tensor_tensor(
                        out=rnv[:, :, 0:HD], in0=hsl,
                        in1=nqcol[:].unsqueeze(2).to_broadcast([P, NT, HD]), op=OP.mult,
                    )
                    nc.vector.tensor_tensor(
                        out=rnv[:, :, NAUG : NAUG + HD], in0=hsl,
                        in1=vcol[:].unsqueeze(2).to_broadcast([P, NT, HD]), op=OP.mult,
                    )
                    nc.vector.tensor_copy(
                        rq[:, :, HD : HD + 1].rearrange("p t one -> p (t one)"), qcol[:]
                    )
                    nc.vector.tensor_copy(
                        rnv[:, :, HD : HD + 1].rearrange("p t one -> p (t one)"), nqcol[:]
                    )
                    nc.vector.tensor_copy(
                        rnv[:, :, 2 * NAUG - 1 : 2 * NAUG].rearrange("p t one -> p (t one)"), vcol[:]
                    )
                    return st

                def mm_row(st, t, cg_lo, cg_hi, aH, first_t):
                    """MMs for mask row t over column groups [cg_lo, cg_hi)."""
                    rq, rnv = st["rq"], st["rnv"]
                    for cg in range(cg_lo, cg_hi):
                        po = po_slice(cg)
                        # start=True marks the WHOLE 2KB psum bank pending-zero,
                        # so only the first group touching each bank may set it.
                        nc.tensor.matmul(
                            po, aH[:, (cg - cg_lo) * P : (cg - cg_lo + 1) * P],
                            rnv[:, t, :],
                            start=(first_t and cg % 3 == 0), stop=(t == NT - 1),
                            skip_group_check=True,
                        )
                        nc.tensor.matmul(
                            po[:, 0:NAUG], adjT[:, t, cg * P : (cg + 1) * P], rq[:, t, :],
                            start=False, stop=(t == NT - 1),
                            skip_group_check=True,
                        )

                def build_row(st, t, i0, i1, tag):
                    bcast, nadst = st["bcast"], st["nadst"]
                    sg = ringp.tile([P, i1 - i0], BF16, tag="sg" + tag, name="sg")
                    nc.vector.tensor_scalar(
                        out=sg[:], in0=bcast[:, i0:i1],
                        scalar1=nadst[:, t : t + 1], scalar2=None, op0=OP.is_ge,
                    )
                    aH = ringp.tile([P, i1 - i0], BF16, tag="adjH" + tag, name="adjH")
                    nc.vector.tensor_tensor(
                        out=aH[:], in0=adjT[:, t, i0:i1], in1=sg[:], op=OP.mult
                    )
                    return aH

                def epilogue_scale(h, st):
                    """PSUM evac + combine (ACT + gpsimd only — no DVE ops)."""
                    ucol, pcol = st["ucol"], st["pcol"]
                    s_all = sallp.tile([P, NT, NAUG], F32, tag="s_all")
                    for cg in range(NT):
                        po = po_slice(cg)
                        e1 = estp.tile([P, 2, NAUG], F32, tag="e1", name="e1")
                        nc.scalar.activation(
                            e1[:, 0, :], po[:, 0:NAUG], AF.Identity,
                            bias=0.0, scale=pcol[:, cg : cg + 1],
                        )
                        nc.scalar.activation(
                            e1[:, 1, :], po[:, NAUG : 2 * NAUG], AF.Identity,
                            bias=0.0, scale=ucol[:, cg : cg + 1],
                        )
                        nc.gpsimd.tensor_tensor(
                            out=s_all[:, cg, :], in0=e1[:, 0, :], in1=e1[:, 1, :], op=OP.add
                        )
                    return s_all

                def epilogue_out(h, s_all):
                    """Normalize + store (DVE + gpsimd) — emitted late so the
                    DVE ops don't head-of-line-block the next head's masks."""
                    rall = estp.tile([P, NT], F32, tag="rall")
                    nc.vector.reciprocal(
                        rall[:], s_all[:, :, HD : HD + 1].rearrange("p t one -> p (t one)")
                    )
                    out_stage = outp.tile([P, NT, HD], F32, tag="outst")
                    for cg in range(NT):
                        nc.vector.tensor_scalar(
                            out=out_stage[:, cg, :], in0=s_all[:, cg, 0:HD],
                            scalar1=rall[:, cg : cg + 1], scalar2=None, op0=OP.mult,
                        )
                    nc.gpsimd.dma_start(
                        out=out_d[:].rearrange("(c p) (hh d) -> p c hh d", p=P, d=HD)[:, :, h, :],
                        in_=out_stage[:],
                    )

                # ---- head 0: split into i-halves so the matmuls can start
                # after only half the adjacency transposes have landed ----
                st = head_prep(0)
                states = {0: st}
                for half in range(2):
                    i0 = half * (L // 2)
                    for t in range(NT):
                        aH = build_row(st, t, i0, i0 + L // 2, "h0")
                        mm_row(st, t, half * 8, half * 8 + 8, aH, first_t=(t == 0))
                        if half == 1 and t == 4:
                            states[1] = head_prep(1)
                pending = (0, epilogue_scale(0, st))

                # ---- heads 1..3: full-row masks ----
                for h in range(1, HEADS):
                    st = states[h]
                    for t in range(NT):
                        aH = build_row(st, t, 0, L, "")
                        mm_row(st, t, 0, NT, aH, first_t=(t == 0))
                        if t == 1 and pending is not None:
                            epilogue_out(*pending)
                            pending = None
                        if t == 4 and h + 1 < HEADS:
                            states[h + 1] = head_prep(h + 1)
                    pending = (h, epilogue_scale(h, st))
                epilogue_out(*pending)

    nc.finalize()
    return nc


def kernel(x, adj_mask, W, att_src, att_dst):
    if "nc" not in _NC_CACHE:
        _NC_CACHE["nc"] = _build()
    nc = _NC_CACHE["nc"]

    x = np.asarray(x, dtype=np.float32)
    W = np.ascontiguousarray(np.asarray(W, dtype=np.float32))
    att_src = np.ascontiguousarray(np.asarray(att_src, dtype=np.float32))
    att_dst = np.ascontiguousarray(np.asarray(att_dst, dtype=np.float32))
    adj_bf = np.asarray(adj_mask).astype(ml_dtypes.bfloat16)
    wT_bf = np.ascontiguousarray(W.T.astype(ml_dtypes.bfloat16))

    in_maps = [
        {
            "xTbf": np.ascontiguousarray(x[b].T.astype(ml_dtypes.bfloat16)),
            "adjbf": np.ascontiguousarray(adj_bf[b]),
            "W": W,
            "wTbf": wT_bf,
            "att_src": att_src,
            "att_dst": att_dst,
        }
        for b in range(N_CORES)
    ]
    res = run_bass_kernel_spmd(nc, in_maps, core_ids=list(range(N_CORES)))
    out = np.stack([res.results[b]["out"] for b in range(N_CORES)], axis=0)
    return out.astype(np.float32)


# revision 8
# speedup vs baseline: 1.1747x; 1.1747x over previous
"""DenseGAT Trainium2 kernel (8 NeuronCores, batch-parallel), v2.

Math: per (batch, head):
  h = x @ W.T ; a_src[i] = h[i]*att_src ; a_dst[j] = h[j]*att_dst
  s_ij = a_src[i] + a_dst[j] ; P = adj * exp(leakyrelu_0.2(s))
  out[i] = (P @ h)[i] / sum_j P[i,j]

Identity: exp(lrelu(s)) = [s>=0]*u_i*v_j + [s<0]*p_i*q_j with
  u = exp(a_src), v = exp(a_dst), p = exp(0.2 a_src), q = exp(0.2 a_dst).
With adjH = adj * [s>=0], rv = v*[h|1], rq = q*[h|1]:
  out_aug = p .* (adjT@rq + adjH@(-rq)) + u .* (adjH@rv)
where col 64 of the aug matmuls is the softmax denominator.

v2 structure vs baseline:
  - host ships adj as bf16 -> adjT built by 16 DMA-xbar transposes straight
    from DRAM (no PE transposes, no PSUM evacuations)
  - host ships x pre-transposed (bf16) and W in both layouts
  - t-outer / cg-inner attention loop with all 16 po accumulators resident
    in PSUM (packed 3 per bank, 6 banks) -> adjH liveness is a small ring of
    full rows, masks are built with full-row [128, 2048] DVE ops
  - rq/rnv built with broadcast tensor_tensor on DVE instead of 48 tiny
    ACT ops per head

Each core handles one batch sample (B=8 across 8 cores).
"""

import numpy as np
import ml_dtypes

import concourse.bass as bass
import concourse.mybir as mybir
import concourse.tile as tile
from concourse import bacc
from concourse.bass_utils import run_bass_kernel_spmd
from concourse.masks import make_identity

P = 128
B, L, CIN, COUT, HEADS = 8, 2048, 256, 256, 4
HD = COUT // HEADS          # 64
NT = L // P                 # 16 tiles along L
KB = CIN // P               # 2 chunks along cin/cout
NEG = 0.2
N_CORES = 8
NAUG = HD + 1               # 65

F32 = mybir.dt.float32
BF16 = mybir.dt.bfloat16
AF = mybir.ActivationFunctionType
OP = mybir.AluOpType

_NC_CACHE = {}


def _build():
    nc = bacc.Bacc(None, target_bir_lowering=False, debug=False)
    adj_in = nc.declare_dram_parameter("adjbf", [L, L], BF16, isOutput=False)
    xt_in = nc.declare_dram_parameter("xTbf", [CIN, L], BF16, isOutput=False)
    w_in = nc.declare_dram_parameter("W", [COUT, CIN], F32, isOutput=False)
    wt_in = nc.declare_dram_parameter("wTbf", [CIN, COUT], BF16, isOutput=False)
    asrc_in = nc.declare_dram_parameter("att_src", [1, HEADS, 1, HD], F32, isOutput=False)
    adst_in = nc.declare_dram_parameter("att_dst", [1, HEADS, 1, HD], F32, isOutput=False)
    out_d = nc.declare_dram_parameter("out", [L, COUT], F32, isOutput=True)

    with tile.TileContext(nc) as tc:
        with (
            tc.tile_pool(name="const", bufs=1) as cpool,
            tc.tile_pool(name="big", bufs=1) as big,
        ):
            ident_f32 = cpool.tile([P, P], F32)
            make_identity(nc, ident_f32)
            ones_bf = cpool.tile([1, P], BF16)
            nc.vector.memset(ones_bf[:], 1.0)

            # persistent tensors
            adjT = big.tile([P, NT, L], BF16)       # adj^T: adjT[j, t, i] = adj[i, t*128+j]
            h_bf = big.tile([P, NT, COUT], BF16)    # h natural (L on partitions)
            a_bf = big.tile([8, L], BF16)           # 2H score rows
            a_cols = big.tile([P, NT, 8], F32)      # transposed score columns

            # ---------------- prep: x/W/scores/h ----------------
            with (
                tc.tile_pool(name="prep", bufs=1) as prep,
                tc.tile_pool(name="prep_ps", bufs=2, space="PSUM") as pps,
                tc.tile_pool(name="small_ps", bufs=2, space="PSUM") as sps,
            ):
                xT_bf = prep.tile([P, KB, L], BF16)
                wT_bf = prep.tile([P, KB, COUT], BF16)
                w_nat = prep.tile([P, KB, CIN], F32)
                attW = prep.tile([P, KB, 2 * HEADS], F32)
                attc = prep.tile([P, KB, 2 * HEADS], BF16)
                a_all = prep.tile([8, L], F32)

                nc.sync.dma_start(
                    out=xT_bf[:], in_=xt_in[:].rearrange("(kb p) l -> p kb l", p=P)
                )
                nc.sync.dma_start(
                    out=wT_bf[:], in_=wt_in[:].rearrange("(kb p) c -> p kb c", p=P)
                )
                nc.sync.dma_start(
                    out=w_nat[:], in_=w_in[:].rearrange("(kb p) c -> p kb c", p=P)
                )
                nc.vector.memset(attW[:], 0.0)
                for h in range(HEADS):
                    cb, prow = divmod(HD * h, P)
                    nc.sync.dma_start(
                        out=attW[prow : prow + HD, cb, 2 * h : 2 * h + 1],
                        in_=asrc_in[0, h, 0, :].rearrange("(d one) -> d one", one=1),
                    )
                    nc.sync.dma_start(
                        out=attW[prow : prow + HD, cb, 2 * h + 1 : 2 * h + 2],
                        in_=adst_in[0, h, 0, :].rearrange("(d one) -> d one", one=1),
                    )
                # adjacency transpose: straight from DRAM through the DMA xbar.
                # These occupy the sync queue for a long time, so they go after
                # every small DMA the prep compute path depends on.
                for c in range(NT):
                    nc.sync.dma_start_transpose(
                        out=adjT[:, :, c * P : (c + 1) * P],
                        in_=adj_in[c * P : (c + 1) * P, :],
                    )

                # attc = W^T @ attW : [cin, 2H] (bf16 for the score matmul)
                for mb in range(KB):
                    ap_ps = sps.tile([P, 2 * HEADS], F32, tag="small")
                    for cb in range(KB):
                        nc.tensor.matmul(
                            ap_ps[:], w_nat[:, cb, mb * P : (mb + 1) * P], attW[:, cb, :],
                            start=(cb == 0), stop=(cb == KB - 1),
                        )
                    nc.scalar.activation(attc[:, mb, :], ap_ps[:], AF.Copy, bias=0.0, scale=1.0)

                # a_all = attc^T @ xT : [2H, L]
                for nb in range(4):
                    a_ps = sps.tile([8, 512], F32, tag="small")
                    for kb in range(KB):
                        nc.tensor.matmul(
                            a_ps[:], attc[:, kb, :], xT_bf[:, kb, nb * 512 : (nb + 1) * 512],
                            start=(kb == 0), stop=(kb == KB - 1),
                        )
                    nc.scalar.activation(
                        a_all[:, nb * 512 : (nb + 1) * 512], a_ps[:], AF.Copy, bias=0.0, scale=1.0
                    )
                nc.vector.tensor_copy(a_bf[:], a_all[:])

                for t in range(NT):
                    acp = sps.tile([P, 8], F32, tag="small")
                    nc.tensor.transpose(
                        acp[:], a_all[0:8, t * P : (t + 1) * P], ident_f32[0:8, 0:8]
                    )
                    nc.scalar.activation(a_cols[:, t, :], acp[:], AF.Copy, bias=0.0, scale=1.0)

                # h = x @ W.T
                for c in range(NT):
                    hp = pps.tile([P, COUT], F32, tag="prep")
                    for kb in range(KB):
                        nc.tensor.matmul(
                            hp[:], xT_bf[:, kb, c * P : (c + 1) * P], wT_bf[:, kb, :],
                            start=(kb == 0), stop=(kb == KB - 1),
                        )
                    nc.scalar.activation(h_bf[:, c, :], hp[:], AF.Copy, bias=0.0, scale=1.0)

            # ---------------- main: per-head attention ----------------
            with (
                tc.tile_pool(name="cols", bufs=2) as colp,
                tc.tile_pool(name="rhs", bufs=2) as rhsp,
                tc.tile_pool(name="bc", bufs=2) as bcp,
                tc.tile_pool(name="ring", bufs=6) as ringp,
                tc.tile_pool(name="est", bufs=6) as estp,
                tc.tile_pool(name="sall", bufs=2) as sallp,
                tc.tile_pool(name="outst", bufs=2) as outp,
                tc.tile_pool(name="po_ps", bufs=1, space="PSUM") as pops,
                tc.tile_pool(name="bc_ps", bufs=2, space="PSUM") as bcps,
            ):
                # one PSUM tile per bank so cross-head WAR dependencies are
                # bank-granular (three cg groups packed per 2KB bank)
                po_banks = [pops.tile([P, 512], F32, tag=f"pob{b}", name=f"pob{b}") for b in range(6)]

                def po_slice(cg):
                    b, o = divmod(cg, 3)
                    return po_banks[b][:, o * 130 : o * 130 + 130]

                def head_prep(h):
                    st = {}
                    asl = a_cols[:, :, 2 * h : 2 * h + 1].rearrange("p t one -> p (t one)")
                    adl = a_cols[:, :, 2 * h + 1 : 2 * h + 2].rearrange("p t one -> p (t one)")
                    # f32 cols for ACT scales / TS scalars
                    ucol = st["ucol"] = colp.tile([P, NT], F32, tag="ucol", name="ucol")
                    pcol = st["pcol"] = colp.tile([P, NT], F32, tag="pcol", name="pcol")
                    nadst = st["nadst"] = colp.tile([P, NT], F32, tag="nadst", name="nadst")
                    # bf16 cols for DVE broadcast multiplies
                    vcol = st["vcol"] = colp.tile([P, NT], BF16, tag="vcol", name="vcol")
                    qcol = st["qcol"] = colp.tile([P, NT], BF16, tag="qcol", name="qcol")
                    nqcol = st["nqcol"] = colp.tile([P, NT], BF16, tag="nqcol", name="nqcol")
                    nc.scalar.activation(ucol[:], asl, AF.Exp, bias=0.0, scale=1.0)
                    nc.scalar.activation(pcol[:], asl, AF.Exp, bias=0.0, scale=NEG)
                    nc.scalar.activation(vcol[:], adl, AF.Exp, bias=0.0, scale=1.0)
                    nc.scalar.activation(qcol[:], adl, AF.Exp, bias=0.0, scale=NEG)
                    nc.vector.tensor_scalar(
                        out=nadst[:], in0=adl, scalar1=-1.0, scalar2=None, op0=OP.mult
                    )
                    nc.vector.tensor_scalar(
                        out=nqcol[:], in0=qcol[:], scalar1=-1.0, scalar2=None, op0=OP.mult
                    )

                    # a_src broadcast tile [128, L] bf16 via rank-1 matmul
                    arow = bcp.tile([1, L], BF16, tag="arow")
                    nc.sync.dma_start(out=arow[:], in_=a_bf[2 * h : 2 * h + 1, :])
                    bcast = st["bcast"] = bcp.tile([P, L], BF16, tag="bcast", name="bcast")
                    for nb in range(4):
                        bps = bcps.tile([P, 512], F32, tag="bps")
                        nc.tensor.matmul(
                            bps[:], ones_bf[:], arow[0:1, nb * 512 : (nb + 1) * 512],
                            start=True, stop=True,
                        )
                        nc.scalar.activation(
                            bcast[:, nb * 512 : (nb + 1) * 512], bps[:], AF.Copy, bias=0.0, scale=1.0
                        )

                    # rhs tensors: rq = q*[h|1]; rnv = [-q*[h|1] | v*[h|1]] packed
                    rq = st["rq"] = rhsp.tile([P, NT, NAUG], BF16, tag="rq", name="rq")
                    rnv = st["rnv"] = rhsp.tile([P, NT, 2 * NAUG], BF16, tag="rnv", name="rnv")
                    hsl = h_bf[:, :, h * HD : (h + 1) * HD]
                    nc.vector.tensor_tensor(
                        out=rq[:, :, 0:HD], in0=hsl,
                        in1=qcol[:].unsqueeze(2).to_broadcast([P, NT, HD]), op=OP.mult,
                    )
                    nc.vector.tensor_tensor(
                        out=rnv[:, :, 0:HD], in0=hsl,
                        in1=nqcol[:].unsqueeze(2).to_broadcast([P, NT, HD]), op=OP.mult,
                    )
                    nc.vector.tensor_tensor(
                        out=rnv[:, :, NAUG : NAUG + HD], in0=hsl,
                        in1=vcol[:].unsqueeze(2).to_broadcast([P, NT, HD]), op=OP.mult,
                    )
                    nc.vector.tensor_copy(
                        rq[:, :, HD : HD + 1].rearrange("p t one -> p (t one)"), qcol[:]
                    )
                    nc.vector.tensor_copy(
                        rnv[:, :, HD : HD + 1].rearrange("p t one -> p (t one)"), nqcol[:]
                    )
                    nc.vector.tensor_copy(
                        rnv[:, :, 2 * NAUG - 1 : 2 * NAUG].rearrange("p t one -> p (t one)"), vcol[:]
                    )
                    return st

                def mm_row(st, t, cg_lo, cg_hi, aH, first_t):
                    """MMs for mask row t over column groups [cg_lo, cg_hi)."""
                    rq, rnv = st["rq"], st["rnv"]
                    for cg in range(cg_lo, cg_hi):
                        po = po_slice(cg)
                        # start=True marks the WHOLE 2KB psum bank pending-zero,
                        # so only the first group touching each bank may set it.
                        nc.tensor.matmul(
                            po, aH[:, (cg - cg_lo) * P : (cg - cg_lo + 1) * P],
                            rnv[:, t, :],
                            start=(first_t and cg % 3 == 0), stop=(t == NT - 1),
                            skip_group_check=True,
                        )
                        nc.tensor.matmul(
                            po[:, 0:NAUG], adjT[:, t, cg * P : (cg + 1) * P], rq[:, t, :],
                            start=False, stop=(t == NT - 1),
                            skip_group_check=True,
                        )

                def build_row(st, t, i0, i1, tag):
                    bcast, nadst = st["bcast"], st["nadst"]
                    sg = ringp.tile([P, i1 - i0], BF16, tag="sg" + tag, name="sg")
                    nc.vector.tensor_scalar(
                        out=sg[:], in0=bcast[:, i0:i1],
                        scalar1=nadst[:, t : t + 1], scalar2=None, op0=OP.is_ge,
                    )
                    aH = ringp.tile([P, i1 - i0], BF16, tag="adjH" + tag, name="adjH")
                    nc.vector.tensor_tensor(
                        out=aH[:], in0=adjT[:, t, i0:i1], in1=sg[:], op=OP.mult
                    )
                    return aH

                def epilogue_scale(h, st):
                    """PSUM evac + combine (ACT + gpsimd only — no DVE ops)."""
                    ucol, pcol = st["ucol"], st["pcol"]
                    s_all = sallp.tile([P, NT, NAUG], F32, tag="s_all")
                    for cg in range(NT):
                        po = po_slice(cg)
                        e1 = estp.tile([P, 2, NAUG], F32, tag="e1", name="e1")
                        nc.scalar.activation(
                            e1[:, 0, :], po[:, 0:NAUG], AF.Identity,
                            bias=0.0, scale=pcol[:, cg : cg + 1],
                        )
                        nc.scalar.activation(
                            e1[:, 1, :], po[:, NAUG : 2 * NAUG], AF.Identity,
                            bias=0.0, scale=ucol[:, cg : cg + 1],
                        )
                        nc.gpsimd.tensor_tensor(
                            out=s_all[:, cg, :], in0=e1[:, 0, :], in1=e1[:, 1, :], op=OP.add
                        )
                    return s_all

                def epilogue_out(h, s_all):
                    """Normalize + store (DVE + gpsimd) — emitted late so the
                    DVE ops don't head-of-line-block the next head's masks."""
                    rall = estp.tile([P, NT], F32, tag="rall")
                    nc.vector.reciprocal(
                        rall[:], s_all[:, :, HD : HD + 1].rearrange("p t one -> p (t one)")
                    )
                    out_stage = outp.tile([P, NT, HD], F32, tag="outst")
                    for cg in range(NT):
                        nc.vector.tensor_scalar(
                            out=out_stage[:, cg, :], in0=s_all[:, cg, 0:HD],
                            scalar1=rall[:, cg : cg + 1], scalar2=None, op0=OP.mult,
                        )
                    nc.gpsimd.dma_start(
                        out=out_d[:].rearrange("(c p) (hh d) -> p c hh d", p=P, d=HD)[:, :, h, :],
                        in_=out_stage[:],
                    )

                # ---- head 0: split into i-halves so the matmuls can start
                # after only half the adjacency transposes have landed ----
                st = head_prep(0)
                states = {0: st}
                for half in range(2):
                    i0 = half * (L // 2)
                    for t in range(NT):
                        aH = build_row(st, t, i0, i0 + L // 2, "h0")
                        mm_row(st, t, half * 8, half * 8 + 8, aH, first_t=(t == 0))
                        if half == 1 and t == 4:
                            states[1] = head_prep(1)
                pending = (0, epilogue_scale(0, st))

                # ---- heads 1..3: full-row masks ----
                for h in range(1, HEADS):
                    st = states[h]
                    for t in range(NT):
                        aH = build_row(st, t, 0, L, "")
                        mm_row(st, t, 0, NT, aH, first_t=(t == 0))
                        if t == 1 and pending is not None:
                            epilogue_out(*pending)
                            pending = None
                        if t == 4 and h + 1 < HEADS:
                            states[h + 1] = head_prep(h + 1)
                    pending = (h, epilogue_scale(h, st))
                epilogue_out(*pending)

    nc.finalize()
    return nc


def kernel(x, adj_mask, W, att_src, att_dst):
    if "nc" not in _NC_CACHE:
        _NC_CACHE["nc"] = _build()
    nc = _NC_CACHE["nc"]

    x = np.asarray(x, dtype=np.float32)
    W = np.ascontiguousarray(np.asarray(W, dtype=np.float32))
    att_src = np.ascontiguousarray(np.asarray(att_src, dtype=np.float32))
    att_dst = np.ascontiguousarray(np.asarray(att_dst, dtype=np.float32))
    adj_bf = np.asarray(adj_mask).astype(ml_dtypes.bfloat16)
    wT_bf = np.ascontiguousarray(W.T.astype(ml_dtypes.bfloat16))

    in_maps = [
        {
            "xTbf": np.ascontiguousarray(x[b].T.astype(ml_dtypes.bfloat16)),
            "adjbf": np.ascontiguousarray(adj_bf[b]),
            "W": W,
            "wTbf": wT_bf,
            "att_src": att_src,
            "att_dst": att_dst,
        }
        for b in range(N_CORES)
    ]
    res = run_bass_kernel_spmd(nc, in_maps, core_ids=list(range(N_CORES)))
    out = np.stack([res.results[b]["out"] for b in range(N_CORES)], axis=0)
    return out.astype(np.float32)
